# revision 1
# baseline (speedup 1.0000x reference)
"""Chamfer-distance loss (nn_CDLoss) on 8 Trainium2 NeuronCores.

Strategy (data parallel over graphs, 2 graphs per core):
  - Host: densify per-graph point clouds from (pred, target, batch) and encode
    each cloud twice in a 5-row layout so ONE K=5 matmul produces squared
    pairwise distances directly:
        row-enc point p : ( x0, x1, x2, ||p||^2, 1 )
        col-enc point q : (-2q0,-2q1,-2q2, 1, ||q||^2 )
        (row.T @ col)[p,q] = ||p-q||^2
    Padding:  fake rows -> all zeros  (their row-min is 0, adds nothing)
              one zero-point column represents ALL to_dense_batch zero pads
              fake cols -> (0,0,0,0,BIG) (never the min for a real row)
  - Device (per core): for each of its 2 graphs and both chamfer directions,
    tile the distance matrix through PSUM (128-row x 512-col matmuls, 2048-col
    PSUM chunks), row-min reduce on the vector engine, then per-lane sums.
  - Host: sum the 8 cores' [128, 2*GPC] partial sums, divide by G*n_max.
"""

import math

import numpy as np

BIG = 1e30
N_CORES = 8


# --------------------------------------------------------------------------
# Device kernel
# --------------------------------------------------------------------------

def build_nc(P: int, gpc: int):
    """Build + compile the per-core Bass/Tile kernel.

    P   : padded points per cloud (multiple of 128)
    gpc : graphs per core
    Inputs  rowx, colx, rowy, coly : [gpc, 5, P] f32
    Output  out : [128, 2*gpc] f32 — per-lane sums of row-mins, one column per
            (graph, direction).
    """
    import concourse.bass as bass
    import concourse.mybir as mybir
    from concourse import bacc, tile

    f32 = mybir.dt.float32
    T = P // 128
    # PSUM chunking of the column axis: <=2048 f32 (4 banks), 512 per matmul
    chunks = []
    c0 = 0
    while c0 < P:
        w = min(2048, P - c0)
        chunks.append((c0, w))
        c0 += w
    n_ch = len(chunks)

    nc = bacc.Bacc("TRN2", target_bir_lowering=False, debug=False)

    rowx = nc.dram_tensor("rowx", [gpc, 5, P], f32, kind="ExternalInput")
    colx = nc.dram_tensor("colx", [gpc, 5, P], f32, kind="ExternalInput")
    rowy = nc.dram_tensor("rowy", [gpc, 5, P], f32, kind="ExternalInput")
    coly = nc.dram_tensor("coly", [gpc, 5, P], f32, kind="ExternalInput")
    out = nc.dram_tensor("out", [128, 2 * gpc], f32, kind="ExternalOutput")

    with tile.TileContext(nc) as tc:
        with (
            tc.tile_pool(name="enc", bufs=2) as enc_pool,
            tc.tile_pool(name="mins", bufs=2) as min_pool,
            tc.tile_pool(name="res", bufs=1) as res_pool,
            tc.tile_pool(name="ps", bufs=2, space="PSUM") as ps_pool,
        ):
            out_sb = res_pool.tile([128, 2 * gpc], f32, name="out_sb")

            pairs = []
            for g in range(gpc):
                pairs.append((rowx[g], coly[g]))  # cham_x direction
                pairs.append((rowy[g], colx[g]))  # cham_y direction

            for pi, (row_dram, col_dram) in enumerate(pairs):
                row_sb = enc_pool.tile([5, P], f32, name="row_sb", tag="row")
                col_sb = enc_pool.tile([5, P], f32, name="col_sb", tag="col")
                nc.sync.dma_start(row_sb[:], row_dram)
                nc.sync.dma_start(col_sb[:], col_dram)

                rowmins = min_pool.tile([128, T], f32, name="rowmins", tag="rm")
                for i in range(T):
                    lhsT = row_sb[:, i * 128:(i + 1) * 128]
                    cmins = min_pool.tile([128, n_ch], f32, name="cmins", tag="cm")
                    for ci, (cstart, w) in enumerate(chunks):
                        ps = ps_pool.tile([128, w], f32, name="ps", tag="ps")
                        for j in range(0, w, 512):
                            n = min(512, w - j)
                            nc.tensor.matmul(
                                ps[:, j:j + n],
                                lhsT,
                                col_sb[:, cstart + j:cstart + j + n],
                            )
                        nc.vector.tensor_reduce(
                            cmins[:, ci:ci + 1], ps[:, :w],
                            axis=mybir.AxisListType.X, op=mybir.AluOpType.min,
                        )
                    nc.vector.tensor_reduce(
                        rowmins[:, i:i + 1], cmins[:],
                        axis=mybir.AxisListType.X, op=mybir.AluOpType.min,
                    )
                nc.vector.reduce_sum(
                    out_sb[:, pi:pi + 1], rowmins[:], axis=mybir.AxisListType.X,
                )

            nc.sync.dma_start(out[:], out_sb[:])

    nc.compile()
    return nc


# --------------------------------------------------------------------------
# Host-side encode / shard / gather
# --------------------------------------------------------------------------

def _encode(v: np.ndarray, c: int, P: int, n_max: int):
    """v: [c,3] points of one cloud. Returns (row_enc [5,P], col_enc [5,P])."""
    row = np.zeros((5, P), np.float32)
    col = np.zeros((5, P), np.float32)
    if c:
        vT = v.T.astype(np.float32)
        sq = (v * v).sum(1, dtype=np.float32)
        row[0:3, :c] = vT
        row[3, :c] = sq
        row[4, :c] = 1.0
        col[0:3, :c] = -2.0 * vT
        col[3, :c] = 1.0
        col[4, :c] = sq
    k = c
    if c < n_max:
        col[3, k] = 1.0  # one zero-point column stands in for all zero pads
        k += 1
    col[4, k:] = BIG  # alignment pad columns: huge distance for real rows
    return row, col


def prepare(pred, target, batch):
    """Returns (in_maps, num_graphs, n_max, P, gpc)."""
    pred = np.ascontiguousarray(np.asarray(pred), dtype=np.float32)
    target = np.ascontiguousarray(np.asarray(target), dtype=np.float32)
    batch = np.asarray(batch).astype(np.int64)

    num_graphs = int(batch.max()) + 1
    counts = np.bincount(batch, minlength=num_graphs)
    n_max = int(counts.max())
    P = ((n_max + 127) // 128) * 128
    gpc = max(1, math.ceil(num_graphs / N_CORES))
    starts = np.zeros(num_graphs + 1, np.int64)
    np.cumsum(counts, out=starts[1:])

    in_maps = []
    for core in range(N_CORES):
        m = {k: np.zeros((gpc, 5, P), np.float32)
             for k in ("rowx", "colx", "rowy", "coly")}
        for slot in range(gpc):
            g = core * gpc + slot
            if g >= num_graphs:
                # unused slot: all-zero rows + BIG cols -> contributes 0
                m["coly"][slot, 4, :] = BIG
                m["colx"][slot, 4, :] = BIG
                continue
            c = int(counts[g])
            x = pred[starts[g]:starts[g + 1]]
            y = target[starts[g]:starts[g + 1]]
            m["rowx"][slot], m["colx"][slot] = _encode(x, c, P, n_max)
            m["rowy"][slot], m["coly"][slot] = _encode(y, c, P, n_max)
        in_maps.append(m)
    return in_maps, num_graphs, n_max, P, gpc


def run(pred, target, batch, trace=False, **spmd_kwargs):
    """Full pipeline. Returns (loss_scalar, BassKernelResults)."""
    from concourse.bass_utils import run_bass_kernel_spmd

    in_maps, num_graphs, n_max, P, gpc = prepare(pred, target, batch)
    nc = build_nc(P, gpc)
    res = run_bass_kernel_spmd(
        nc, in_maps, core_ids=list(range(N_CORES)), trace=trace, **spmd_kwargs,
    )
    total = 0.0
    for core in range(N_CORES):
        total += res.results[core]["out"].astype(np.float64).sum()
    loss = np.float32(total / (num_graphs * n_max))
    return loss, res


def kernel(pred, target, batch):
    loss, _ = run(pred, target, batch, trace=False)
    return loss


# revision 7
# speedup vs baseline: 2.5176x; 2.5176x over previous
"""Chamfer-distance loss (nn_CDLoss) on 8 Trainium2 NeuronCores.

Strategy (data parallel over graphs, 2 graphs per core):
  - Host: densify per-graph point clouds from (pred, target, batch) and encode
    each cloud twice in a 13-row bf16 layout so ONE K=13 matmul produces
    squared pairwise distances directly in fp32 PSUM. fp32 matmuls on TRN2 run
    in slow LOW_HIGH mode, so we do the hi/lo bf16 split ourselves (dropping
    only the lo*lo cross term, ~1e-6 relative on the loss):
        p = ph + pl (bf16 hi/lo), n_p = ||p||^2 = nh_p + nl_p (bf16 hi/lo)
        row-enc p : ( ph[3], pl[3], ph[3], nh_p, nl_p, 1, 1 )
        col-enc q : (-2qh[3], -2qh[3], -2ql[3], 1, 1, nh_q, nl_q )
        (row.T @ col)[p,q] = -2(ph qh + pl qh + ph ql) + n_p + n_q ~= ||p-q||^2
    Padding:  fake rows -> all zeros  (their row-min is 0, adds nothing)
              one zero-point column represents ALL to_dense_batch zero pads
              fake cols -> BIG in slot 11 (never the min for a real row)
  - Device (per core): for each of its 2 graphs and both chamfer directions,
    tile the distance matrix through PSUM (128-row x 512-col matmuls, 2048-col
    PSUM chunks), row-min reduce on the vector engine, then per-lane sums.
  - Host: sum the 8 cores' [128, 2*GPC] partial sums, divide by G*n_max.
"""

import math

import ml_dtypes
import numpy as np

BF16 = ml_dtypes.bfloat16
BIG = 1e30
K = 13
N_CORES = 8


# --------------------------------------------------------------------------
# Device kernel
# --------------------------------------------------------------------------

def build_nc(P: int, gpc: int):
    """Build + compile the per-core Bass/Tile kernel.

    P   : padded points per cloud (multiple of 128)
    gpc : graphs per core
    Inputs  rowx, colx, rowy, coly : [gpc, K, P] bf16
    Output  out : [128, 2*gpc] f32 — per-lane sums of row-mins, one column per
            (graph, direction).
    """
    import concourse.bass as bass
    import concourse.mybir as mybir
    from concourse import bacc, tile

    f32 = mybir.dt.float32
    bf16 = mybir.dt.bfloat16
    T = P // 128
    # PSUM chunking of the column axis: <=2048 f32 (4 banks), 512 per matmul
    chunks = []
    c0 = 0
    while c0 < P:
        w = min(2048, P - c0)
        chunks.append((c0, w))
        c0 += w
    n_ch = len(chunks)

    nc = bacc.Bacc("TRN2", target_bir_lowering=False, debug=False)

    rowx = nc.dram_tensor("rowx", [gpc, K, P], bf16, kind="ExternalInput")
    colx = nc.dram_tensor("colx", [gpc, K, P], bf16, kind="ExternalInput")
    rowy = nc.dram_tensor("rowy", [gpc, K, P], bf16, kind="ExternalInput")
    coly = nc.dram_tensor("coly", [gpc, K, P], bf16, kind="ExternalInput")
    out = nc.dram_tensor("out", [128, 2 * gpc], f32, kind="ExternalOutput")

    with tile.TileContext(nc) as tc:
        with (
            tc.tile_pool(name="enc", bufs=2) as enc_pool,
            tc.tile_pool(name="mins", bufs=2) as min_pool,
            tc.tile_pool(name="res", bufs=1) as res_pool,
            tc.tile_pool(name="ps", bufs=2, space="PSUM") as ps_pool,
        ):
            out_sb = res_pool.tile([128, 2 * gpc], f32, name="out_sb")

            pairs = []
            for g in range(gpc):
                pairs.append((rowx[g], coly[g]))  # cham_x direction
                pairs.append((rowy[g], colx[g]))  # cham_y direction

            for pi, (row_dram, col_dram) in enumerate(pairs):
                row_sb = enc_pool.tile([K, P], bf16, name="row_sb", tag="row")
                col_sb = enc_pool.tile([K, P], bf16, name="col_sb", tag="col")
                nc.sync.dma_start(row_sb[:], row_dram)
                nc.sync.dma_start(col_sb[:], col_dram)

                # pm[:, i, ci] = min over chunk ci of row tile i
                pm = min_pool.tile([128, T, n_ch], f32, name="pm", tag="pm")
                rowmins = min_pool.tile([128, T], f32, name="rowmins", tag="rm")
                for i in range(T):
                    lhsT = row_sb[:, i * 128:(i + 1) * 128]
                    for ci, (cstart, w) in enumerate(chunks):
                        ps = ps_pool.tile([128, w], f32, name="ps", tag="ps")
                        for j in range(0, w, 512):
                            n = min(512, w - j)
                            nc.tensor.matmul(
                                ps[:, j:j + n],
                                lhsT,
                                col_sb[:, cstart + j:cstart + j + n],
                            )
                        nc.vector.tensor_reduce(
                            pm[:, i, ci:ci + 1], ps[:, :w],
                            axis=mybir.AxisListType.X, op=mybir.AluOpType.min,
                        )
                nc.vector.tensor_reduce(
                    rowmins[:], pm[:],
                    axis=mybir.AxisListType.X, op=mybir.AluOpType.min,
                )
                nc.vector.reduce_sum(
                    out_sb[:, pi:pi + 1], rowmins[:], axis=mybir.AxisListType.X,
                )

            nc.sync.dma_start(out[:], out_sb[:])

    nc.compile()
    return nc


# --------------------------------------------------------------------------
# Host-side encode / shard / gather
# --------------------------------------------------------------------------

def _encode(v: np.ndarray, c: int, P: int, n_max: int):
    """v: [c,3] points of one cloud. Returns (row_enc [K,P], col_enc [K,P]) bf16."""
    row = np.zeros((K, P), np.float32)
    col = np.zeros((K, P), np.float32)
    if c:
        v = v.astype(np.float32)
        vh = v.astype(BF16).astype(np.float32)
        vl = (v - vh).astype(BF16).astype(np.float32)
        n = (v.astype(np.float64) ** 2).sum(1)
        nh = n.astype(BF16).astype(np.float64)
        nl = (n - nh).astype(BF16).astype(np.float32)
        row[0:3, :c] = vh.T
        row[3:6, :c] = vl.T
        row[6:9, :c] = vh.T
        row[9, :c] = nh
        row[10, :c] = nl
        row[11, :c] = 1.0
        row[12, :c] = 1.0
        col[0:3, :c] = -2.0 * vh.T
        col[3:6, :c] = -2.0 * vh.T
        col[6:9, :c] = -2.0 * vl.T
        col[9, :c] = 1.0
        col[10, :c] = 1.0
        col[11, :c] = nh
        col[12, :c] = nl
    k = c
    if c < n_max:
        # one zero-point column stands in for all to_dense_batch zero pads
        col[9, k] = 1.0
        col[10, k] = 1.0
        k += 1
    col[11, k:] = BIG  # alignment pad columns: huge distance for real rows
    return row.astype(BF16), col.astype(BF16)


def prepare(pred, target, batch):
    """Returns (in_maps, num_graphs, n_max, P, gpc)."""
    pred = np.ascontiguousarray(np.asarray(pred), dtype=np.float32)
    target = np.ascontiguousarray(np.asarray(target), dtype=np.float32)
    batch = np.asarray(batch).astype(np.int64)

    num_graphs = int(batch.max()) + 1
    counts = np.bincount(batch, minlength=num_graphs)
    n_max = int(counts.max())
    P = ((n_max + 127) // 128) * 128
    gpc = max(1, math.ceil(num_graphs / N_CORES))
    starts = np.zeros(num_graphs + 1, np.int64)
    np.cumsum(counts, out=starts[1:])

    empty = np.zeros((0, 3), np.float32)
    in_maps = []
    for core in range(N_CORES):
        m = {k: np.zeros((gpc, K, P), BF16)
             for k in ("rowx", "colx", "rowy", "coly")}
        for slot in range(gpc):
            g = core * gpc + slot
            if g >= num_graphs:
                # unused slot: all-zero rows contribute 0 to the sums
                m["rowx"][slot], m["colx"][slot] = _encode(empty, 0, P, n_max)
                m["rowy"][slot], m["coly"][slot] = _encode(empty, 0, P, n_max)
                continue
            c = int(counts[g])
            x = pred[starts[g]:starts[g + 1]]
            y = target[starts[g]:starts[g + 1]]
            m["rowx"][slot], m["colx"][slot] = _encode(x, c, P, n_max)
            m["rowy"][slot], m["coly"][slot] = _encode(y, c, P, n_max)
        in_maps.append(m)
    return in_maps, num_graphs, n_max, P, gpc


def run(pred, target, batch, trace=False, **spmd_kwargs):
    """Full pipeline. Returns (loss_scalar, BassKernelResults)."""
    from concourse.bass_utils import run_bass_kernel_spmd

    in_maps, num_graphs, n_max, P, gpc = prepare(pred, target, batch)
    nc = build_nc(P, gpc)
    res = run_bass_kernel_spmd(
        nc, in_maps, core_ids=list(range(N_CORES)), trace=trace, **spmd_kwargs,
    )
    total = 0.0
    for core in range(N_CORES):
        total += res.results[core]["out"].astype(np.float64).sum()
    loss = np.float32(total / (num_graphs * n_max))
    return loss, res


def kernel(pred, target, batch):
    loss, _ = run(pred, target, batch, trace=False)
    return loss


# revision 10
# speedup vs baseline: 2.5181x; 1.0002x over previous
"""Chamfer-distance loss (nn_CDLoss) on 8 Trainium2 NeuronCores.

Strategy (data parallel over graphs, 2 graphs per core):
  - Host: densify per-graph point clouds from (pred, target, batch) and encode
    each cloud twice in a 13-row bf16 layout so ONE K=13 matmul produces
    squared pairwise distances directly in fp32 PSUM. fp32 matmuls on TRN2 run
    in slow LOW_HIGH mode, so we do the hi/lo bf16 split ourselves (dropping
    only the lo*lo cross term, ~1e-6 relative on the loss):
        p = ph + pl (bf16 hi/lo), n_p = ||p||^2 = nh_p + nl_p (bf16 hi/lo)
        row-enc p : ( ph[3], pl[3], ph[3], nh_p, nl_p, 1, 1 )
        col-enc q : (-2qh[3], -2qh[3], -2ql[3], 1, 1, nh_q, nl_q )
        (row.T @ col)[p,q] = -2(ph qh + pl qh + ph ql) + n_p + n_q ~= ||p-q||^2
    Padding:  fake rows -> all zeros  (their row-min is 0, adds nothing)
              one zero-point column represents ALL to_dense_batch zero pads
              fake cols -> BIG in slot 11 (never the min for a real row)
  - Device (per core): for each of its 2 graphs and both chamfer directions,
    tile the distance matrix through PSUM (128-row x 512-col matmuls, 2048-col
    PSUM chunks), row-min reduce on the vector engine, then per-lane sums.
  - Host: sum the 8 cores' [128, 2*GPC] partial sums, divide by G*n_max.
"""

import math

import ml_dtypes
import numpy as np

BF16 = ml_dtypes.bfloat16
BIG = 1e30
K = 13
N_CORES = 8


# --------------------------------------------------------------------------
# Device kernel
# --------------------------------------------------------------------------

def build_nc(P: int, gpc: int):
    """Build + compile the per-core Bass/Tile kernel.

    P   : padded points per cloud (multiple of 128)
    gpc : graphs per core
    Inputs  rowx, colx, rowy, coly : [gpc, K, P] bf16
    Output  out : [128, 2*gpc] f32 — per-lane sums of row-mins, one column per
            (graph, direction).
    """
    import concourse.bass as bass
    import concourse.mybir as mybir
    from concourse import bacc, tile
    from concourse.tile import add_dep_helper

    f32 = mybir.dt.float32
    bf16 = mybir.dt.bfloat16
    T = P // 128
    # PSUM chunking of the column axis: <=2048 f32 (4 banks), 512 per matmul
    chunks = []
    c0 = 0
    while c0 < P:
        w = min(2048, P - c0)
        chunks.append((c0, w))
        c0 += w
    n_ch = len(chunks)

    nc = bacc.Bacc("TRN2", target_bir_lowering=False, debug=False)

    rowx = nc.dram_tensor("rowx", [gpc, K, P], bf16, kind="ExternalInput")
    colx = nc.dram_tensor("colx", [gpc, K, P], bf16, kind="ExternalInput")
    rowy = nc.dram_tensor("rowy", [gpc, K, P], bf16, kind="ExternalInput")
    coly = nc.dram_tensor("coly", [gpc, K, P], bf16, kind="ExternalInput")
    out = nc.dram_tensor("out", [128, 2 * gpc], f32, kind="ExternalOutput")

    with tile.TileContext(nc) as tc:
        with (
            tc.tile_pool(name="enc", bufs=2) as enc_pool,
            tc.tile_pool(name="mins", bufs=2) as min_pool,
            tc.tile_pool(name="res", bufs=1) as res_pool,
            tc.tile_pool(name="ps", bufs=2, space="PSUM") as ps_pool,
        ):
            out_sb = res_pool.tile([128, 2 * gpc], f32, name="out_sb")
            prev_mm = None

            pairs = []
            for g in range(gpc):
                pairs.append((rowx[g], coly[g]))  # cham_x direction
                pairs.append((rowy[g], colx[g]))  # cham_y direction

            for pi, (row_dram, col_dram) in enumerate(pairs):
                row_sb = enc_pool.tile([K, P], bf16, name="row_sb", tag="row")
                col_sb = enc_pool.tile([K, P], bf16, name="col_sb", tag="col")
                nc.sync.dma_start(row_sb[:], row_dram)
                nc.sync.dma_start(col_sb[:], col_dram)

                # pm[:, i, ci] = min over chunk ci of row tile i
                pm = min_pool.tile([128, T, n_ch], f32, name="pm", tag="pm")
                rowmins = min_pool.tile([128, T], f32, name="rowmins", tag="rm")
                for i in range(T):
                    lhsT = row_sb[:, i * 128:(i + 1) * 128]
                    first_of_tile = True
                    for ci, (cstart, w) in enumerate(chunks):
                        ps = ps_pool.tile([128, w], f32, name="ps", tag="ps")
                        for j in range(0, w, 512):
                            n = min(512, w - j)
                            mm = nc.tensor.matmul(
                                ps[:, j:j + n],
                                lhsT,
                                col_sb[:, cstart + j:cstart + j + n],
                            )
                            # The 9 matmuls of one row tile share lhsT: only
                            # the first self-loads weights. The explicit PE
                            # dep chain pins program order so a weight-reusing
                            # matmul can never run after another tile's load.
                            if not first_of_tile:
                                mm.ins.ldweights = False
                            first_of_tile = False
                            if prev_mm is not None:
                                add_dep_helper(mm.ins, prev_mm.ins, False,
                                               "pe program order")
                            prev_mm = mm
                        nc.vector.tensor_reduce(
                            pm[:, i, ci:ci + 1], ps[:, :w],
                            axis=mybir.AxisListType.X, op=mybir.AluOpType.min,
                        )
                nc.vector.tensor_reduce(
                    rowmins[:], pm[:],
                    axis=mybir.AxisListType.X, op=mybir.AluOpType.min,
                )
                nc.vector.reduce_sum(
                    out_sb[:, pi:pi + 1], rowmins[:], axis=mybir.AxisListType.X,
                )

            nc.sync.dma_start(out[:], out_sb[:])

    nc.compile()
    return nc


# --------------------------------------------------------------------------
# Host-side encode / shard / gather
# --------------------------------------------------------------------------

def _encode(v: np.ndarray, c: int, P: int, n_max: int):
    """v: [c,3] points of one cloud. Returns (row_enc [K,P], col_enc [K,P]) bf16."""
    row = np.zeros((K, P), np.float32)
    col = np.zeros((K, P), np.float32)
    if c:
        v = v.astype(np.float32)
        vh = v.astype(BF16).astype(np.float32)
        vl = (v - vh).astype(BF16).astype(np.float32)
        n = (v.astype(np.float64) ** 2).sum(1)
        nh = n.astype(BF16).astype(np.float64)
        nl = (n - nh).astype(BF16).astype(np.float32)
        row[0:3, :c] = vh.T
        row[3:6, :c] = vl.T
        row[6:9, :c] = vh.T
        row[9, :c] = nh
        row[10, :c] = nl
        row[11, :c] = 1.0
        row[12, :c] = 1.0
        col[0:3, :c] = -2.0 * vh.T
        col[3:6, :c] = -2.0 * vh.T
        col[6:9, :c] = -2.0 * vl.T
        col[9, :c] = 1.0
        col[10, :c] = 1.0
        col[11, :c] = nh
        col[12, :c] = nl
    k = c
    if c < n_max:
        # one zero-point column stands in for all to_dense_batch zero pads
        col[9, k] = 1.0
        col[10, k] = 1.0
        k += 1
    col[11, k:] = BIG  # alignment pad columns: huge distance for real rows
    return row.astype(BF16), col.astype(BF16)


def prepare(pred, target, batch):
    """Returns (in_maps, num_graphs, n_max, P, gpc)."""
    pred = np.ascontiguousarray(np.asarray(pred), dtype=np.float32)
    target = np.ascontiguousarray(np.asarray(target), dtype=np.float32)
    batch = np.asarray(batch).astype(np.int64)

    num_graphs = int(batch.max()) + 1
    counts = np.bincount(batch, minlength=num_graphs)
    n_max = int(counts.max())
    P = ((n_max + 127) // 128) * 128
    gpc = max(1, math.ceil(num_graphs / N_CORES))
    starts = np.zeros(num_graphs + 1, np.int64)
    np.cumsum(counts, out=starts[1:])

    empty = np.zeros((0, 3), np.float32)
    in_maps = []
    for core in range(N_CORES):
        m = {k: np.zeros((gpc, K, P), BF16)
             for k in ("rowx", "colx", "rowy", "coly")}
        for slot in range(gpc):
            g = core * gpc + slot
            if g >= num_graphs:
                # unused slot: all-zero rows contribute 0 to the sums
                m["rowx"][slot], m["colx"][slot] = _encode(empty, 0, P, n_max)
                m["rowy"][slot], m["coly"][slot] = _encode(empty, 0, P, n_max)
                continue
            c = int(counts[g])
            x = pred[starts[g]:starts[g + 1]]
            y = target[starts[g]:starts[g + 1]]
            m["rowx"][slot], m["colx"][slot] = _encode(x, c, P, n_max)
            m["rowy"][slot], m["coly"][slot] = _encode(y, c, P, n_max)
        in_maps.append(m)
    return in_maps, num_graphs, n_max, P, gpc


def run(pred, target, batch, trace=False, **spmd_kwargs):
    """Full pipeline. Returns (loss_scalar, BassKernelResults)."""
    from concourse.bass_utils import run_bass_kernel_spmd

    in_maps, num_graphs, n_max, P, gpc = prepare(pred, target, batch)
    nc = build_nc(P, gpc)
    res = run_bass_kernel_spmd(
        nc, in_maps, core_ids=list(range(N_CORES)), trace=trace, **spmd_kwargs,
    )
    total = 0.0
    for core in range(N_CORES):
        total += res.results[core]["out"].astype(np.float64).sum()
    loss = np.float32(total / (num_graphs * n_max))
    return loss, res


def kernel(pred, target, batch):
    loss, _ = run(pred, target, batch, trace=False)
    return loss
